# revision 1
# baseline (speedup 1.0000x reference)
"""Cross-attention kernel for TRN2, 8 NeuronCores.

Sharding: core c -> (batch b = c//2, head-group g = c%2).  Each head-group is
8 heads = 512 of the 1024 d_model channels.  Within a core:
  QT = (Wq_g/8) @ q_b.T + bq_g/8        [512, 512]   (s, lq)   scale folded
  KT = Wk_g @ kv_b.T + bk_g             [512, 2048]  (s, lkv)
  V  = kv_b @ Wv_g.T + bv_g             [2048, 512]  (lkv, s)
  ST_h = Kh @ Qh.T                      [2048, 512]  per head (lkv, lq)
  PT_h = exp(ST_h)        (no max-sub: scores ~N(0,1), bounded)
  cT_h = V_h.T @ PT_h / colsum(PT_h)    [64, 512]    (s, lq)
  out_partial = cT.T.T @ Wo_g.T         [512, 1024]  (lq, d)
Host sums the two head-group partials per batch and adds bo.

All matmuls run as float32r (TF32-ish, full PE rate at N=512).  Head pairs are
packed onto the 128-wide PE array via partition-offset row/col tiling.
"""

import sys
if "/opt/trn_rl_repo" not in sys.path:
    sys.path.insert(0, "/opt/trn_rl_repo")

import numpy as np

import concourse.bass as bass
import concourse.mybir as mybir
import concourse.tile as tile
from concourse.bass_utils import run_bass_kernel_spmd

f32 = mybir.dt.float32
f32r = mybir.dt.float32r
EXP = mybir.ActivationFunctionType.Exp
IDENT = mybir.ActivationFunctionType.Identity

D = 1024        # d_model
S = 512         # per-core channel shard (8 heads x 64)
LQ = 512
LKV = 2048
CO = D // 128   # 8 contraction chunks
SO = S // 128   # 4 shard s-tiles
NT = LKV // 128  # 16 lkv tiles
NKC = LKV // 512  # 4 lkv 512-chunks


def _split_multi_waits(nc, max_waits=1):
    """This container's walrus allows only `max_waits` sync-wait commands per
    instruction; hoist the excess into standalone EventSemaphore insts."""
    ev_id = 0
    for f in nc.m.functions:
        for bb in f.blocks:
            new = []
            changed = False
            for inst in bb.instructions:
                si = inst.sync_info
                if si is not None and si.on_wait and len(si.on_wait) > max_waits:
                    waits = list(si.on_wait)
                    for sw in waits[:-max_waits]:
                        ev = mybir.InstEventSemaphore(
                            name=f"EVSPLIT-{ev_id}", engine=inst.engine,
                            sync_info=mybir.SyncInfo(on_wait=[sw], on_update=[]))
                        ev_id += 1
                        nc.register_instruction(ev, overwrite=True)
                        new.append(ev)
                    inst.sync_info = mybir.SyncInfo(
                        on_wait=waits[-max_waits:], on_update=list(si.on_update))
                    changed = True
                new.append(inst)
            if changed:
                bb.instructions = new
    return nc


def _build():
    nc = bass.Bass(trn_type="TRN2")

    # DRAM I/O (activations/weights pre-laid-out [128, outer, free] on host)
    qT = nc.dram_tensor("qT", [128, CO, LQ], f32r, kind="ExternalInput")
    kvT = nc.dram_tensor("kvT", [128, CO, LKV], f32r, kind="ExternalInput")
    wqT = nc.dram_tensor("wqT", [128, CO, S], f32r, kind="ExternalInput")
    wkT = nc.dram_tensor("wkT", [128, CO, S], f32r, kind="ExternalInput")
    wvT = nc.dram_tensor("wvT", [128, CO, S], f32r, kind="ExternalInput")
    woT = nc.dram_tensor("woT", [128, SO, D], f32r, kind="ExternalInput")
    bq = nc.dram_tensor("bq", [128, SO], f32, kind="ExternalInput")
    bk = nc.dram_tensor("bk", [128, SO], f32, kind="ExternalInput")
    bv = nc.dram_tensor("bv", [1, S], f32r, kind="ExternalInput")
    out = nc.dram_tensor("out", [SO, 128, D], f32, kind="ExternalOutput")

    with tile.TileContext(nc) as tc:
        with tc.tile_pool(name="wgt", bufs=1) as wgt, \
             tc.tile_pool(name="big", bufs=1) as big, \
             tc.tile_pool(name="strm", bufs=3) as strm, \
             tc.tile_pool(name="pt", bufs=2) as ptp, \
             tc.tile_pool(name="ostg", bufs=2) as ostg, \
             tc.tile_pool(name="sml", bufs=2) as sml, \
             tc.tile_pool(name="psA", bufs=4, space="PSUM") as psA, \
             tc.tile_pool(name="psC", bufs=2, space="PSUM") as psC:

            # ---- resident weights / constants ----
            wk_sb = wgt.tile([128, CO, S], f32r, name="wk_sb")
            wv_sb = wgt.tile([128, CO, S], f32r, name="wv_sb")
            kv_sb = wgt.tile([128, CO, LKV], f32r, name="kv_sb")
            bq_sb = wgt.tile([128, SO], f32, name="bq_sb")
            bk_sb = wgt.tile([128, SO], f32, name="bk_sb")
            bv_sb = wgt.tile([1, S], f32r, name="bv_sb")
            ones_f = wgt.tile([128, 128], f32, name="ones_f")
            ones = wgt.tile([128, 128], f32r, name="ones")
            for c in range(CO):
                nc.sync.dma_start(wk_sb[:, c, :], wkT[:, c, :])
                nc.sync.dma_start(wv_sb[:, c, :], wvT[:, c, :])
                nc.sync.dma_start(kv_sb[:, c, :], kvT[:, c, :])
            nc.sync.dma_start(bq_sb, bq[:])
            nc.sync.dma_start(bk_sb, bk[:])
            nc.sync.dma_start(bv_sb, bv[:])
            nc.vector.memset(ones_f, 1.0)
            nc.vector.tensor_copy(ones, ones_f)

            # broadcast bv across partitions: ones[1,128].T @ bv[1,512]
            bv_ps = psA.tile([128, S], f32, name="bv_ps", tag="mm")
            nc.tensor.matmul(bv_ps, ones[0:1, :], bv_sb, start=True, stop=True)
            bv_bc = wgt.tile([128, S], f32r, name="bv_bc")
            nc.vector.tensor_copy(bv_bc, bv_ps)

            # ---- resident intermediates ----
            KT_sb = big.tile([128, SO, LKV], f32r, name="KT_sb")   # (s, lkv)
            # V padded per head with a ones column: [lkv, t, head, 64+1].
            # The ones column makes attn@V also produce the softmax
            # denominator as psum row 64 (col-tiling to upper partitions is
            # rejected by this walrus, so no separate denominator matmuls).
            Vp_sb = big.tile([128, NT, 8, 65], f32r, name="Vp_sb")
            QT_sb = big.tile([128, SO, LQ], f32r, name="QT_sb")    # (s, lq)
            cT_sb = big.tile([128, SO, LQ], f32r, name="cT_sb")    # (s, lq)
            nc.vector.tensor_copy(
                Vp_sb[:, :, :, 64:65],
                ones_f[:, 0:128].rearrange("p (a b c) -> p a b c", a=NT, b=8, c=1))

            # ---- K projection: KT[s, lkv] += wk[c,s].T @ kv_sb[c, lkv] ----
            for ch in range(NKC):
                kps = [psA.tile([128, 512], f32, name=f"kps{o}_{ch}", tag="mm")
                       for o in range(SO)]
                for c in range(CO):
                    for o in range(SO):
                        nc.tensor.matmul(
                            kps[o], wk_sb[:, c, o * 128:(o + 1) * 128],
                            kv_sb[:, c, ch * 512:(ch + 1) * 512],
                            start=(c == 0), stop=(c == CO - 1))
                for o in range(SO):
                    nc.scalar.activation(
                        KT_sb[:, o, ch * 512:(ch + 1) * 512], kps[o], IDENT,
                        bias=bk_sb[:, o:o + 1])

            # ---- V projection: V[lkv, s] += kv_sb[c, lkv].T @ wv[c, s] ----
            for t in range(NT):
                vps = psA.tile([128, 512], f32, name="vps", tag="mm")
                for c in range(CO):
                    nc.tensor.matmul(vps, kv_sb[:, c, t * 128:(t + 1) * 128],
                                     wv_sb[:, c, :],
                                     start=(c == 0), stop=(c == CO - 1))
                nc.vector.tensor_add(
                    Vp_sb[:, t, :, 0:64],
                    vps.rearrange("p (h d) -> p h d", h=8),
                    bv_bc.rearrange("p (h d) -> p h d", h=8))

            # ---- Q projection: QT[s, lq] += wq[c,s].T @ qT[c, lq] ----
            qps = [psA.tile([128, 512], f32, name=f"qps{o}", tag="mm")
                   for o in range(SO)]
            for c in range(CO):
                qtt = strm.tile([128, 512], f32r, name="qtt", tag="st512")
                nc.sync.dma_start(qtt, qT[:, c, :])
                wqc = strm.tile([128, S], f32r, name="wqc", tag="st512")
                nc.sync.dma_start(wqc, wqT[:, c, :])
                for o in range(SO):
                    nc.tensor.matmul(
                        qps[o], wqc[:, o * 128:(o + 1) * 128], qtt,
                        start=(c == 0), stop=(c == CO - 1))
            for o in range(SO):
                nc.scalar.activation(QT_sb[:, o, :], qps[o], IDENT,
                                     bias=bq_sb[:, o:o + 1])

            # ---- attention, head pairs (2o, 2o+1) ----
            # scores row-tiled (dh=64 contraction at row offsets 0/64);
            # attn@V per head with M=65 (64 V cols + ones col -> denominator
            # lands in psum row 64).
            for o in range(SO):
                ctxA = psC.tile([65, 512], f32, name="ctxA", tag="ctxA")
                ctxB = psC.tile([65, 512], f32, name="ctxB", tag="ctxB")
                for t in range(NT):
                    stA = psA.tile([128, 512], f32, name="stA", tag="mm")
                    stB = psA.tile([128, 512], f32, name="stB", tag="mm")
                    # S.T tile = Kh[., t-slice].T-contraction over dh=64 rows
                    nc.tensor.matmul(stA, KT_sb[0:64, o, t * 128:(t + 1) * 128],
                                     QT_sb[0:64, o, :], start=True, stop=True)
                    nc.tensor.matmul(stB, KT_sb[64:128, o, t * 128:(t + 1) * 128],
                                     QT_sb[64:128, o, :], start=True, stop=True)
                    ptA = ptp.tile([128, 512], f32r, name="ptA", tag="ptA")
                    ptB = ptp.tile([128, 512], f32r, name="ptB", tag="ptB")
                    nc.scalar.activation(ptA, stA, EXP)
                    nc.scalar.activation(ptB, stB, EXP)
                    st = (t == 0)
                    sp = (t == NT - 1)
                    nc.tensor.matmul(ctxA, Vp_sb[:, t, 2 * o, :], ptA,
                                     start=st, stop=sp)
                    nc.tensor.matmul(ctxB, Vp_sb[:, t, 2 * o + 1, :], ptB,
                                     start=st, stop=sp)
                # normalize: cT_h = ctx_h[0:64] * (1/ctx_h[64]) bcast to 64 rows
                for h, ctx in ((0, ctxA), (1, ctxB)):
                    rc = sml.tile([1, 512], f32r, name="rc", tag="rc")
                    with nc.allow_low_precision(reason="softmax recip f32r"):
                        nc.vector.reciprocal(rc, ctx[64:65, :])
                    nb_ps = psA.tile([64, 512], f32, name="nb_ps", tag="mm")
                    nc.tensor.matmul(nb_ps, ones[0:1, 0:64], rc,
                                     start=True, stop=True)
                    nb_sb = sml.tile([64, 512], f32, name="nb_sb", tag="nb")
                    nc.vector.tensor_copy(nb_sb, nb_ps)
                    nc.vector.tensor_mul(
                        cT_sb[h * 64:(h + 1) * 64, o, :], ctx[0:64, :], nb_sb)

            # ---- out projection: out[lq, d] += cT[s, lq-slice].T @ wo[s, d] ----
            for dc in range(2):
                opss = [psA.tile([128, 512], f32, name=f"ops{lt}", tag="mm")
                        for lt in range(SO)]
                for o in range(SO):
                    woc = strm.tile([128, 512], f32r, name="woc", tag="st512")
                    nc.sync.dma_start(woc, woT[:, o, dc * 512:(dc + 1) * 512])
                    for lt in range(SO):
                        nc.tensor.matmul(
                            opss[lt], cT_sb[:, o, lt * 128:(lt + 1) * 128],
                            woc, start=(o == 0), stop=(o == SO - 1))
                for lt in range(SO):
                    ot = ostg.tile([128, 512], f32, name="ot", tag="ot")
                    nc.vector.tensor_copy(ot, opss[lt])
                    nc.sync.dma_start(out[lt, :, dc * 512:(dc + 1) * 512], ot)

    return _split_multi_waits(nc)


_NC = None


def _get_nc():
    global _NC
    if _NC is None:
        _NC = _build()
    return _NC


def _shard(q, kv, Wq, bq, Wk, bk, Wv, bv, Wo, bo):
    def lay(a2d, co):  # [co*128, F] -> [128, co, F]
        F = a2d.shape[1]
        return np.ascontiguousarray(
            a2d.reshape(co, 128, F).transpose(1, 0, 2))

    in_maps = []
    for core in range(8):
        b, g = core // 2, core % 2
        sl = slice(g * S, (g + 1) * S)
        m = {
            "qT": lay(np.ascontiguousarray(q[b].T), CO),
            "kvT": lay(np.ascontiguousarray(kv[b].T), CO),
            "wqT": lay(np.ascontiguousarray((Wq[sl] * 0.125).T), CO),
            "wkT": lay(np.ascontiguousarray(Wk[sl].T), CO),
            "wvT": lay(np.ascontiguousarray(Wv[sl].T), CO),
            "woT": lay(np.ascontiguousarray(Wo[:, sl].T), SO),
            "bq": np.ascontiguousarray((bq[sl] * 0.125).reshape(SO, 128).T),
            "bk": np.ascontiguousarray(bk[sl].reshape(SO, 128).T),
            "bv": np.ascontiguousarray(bv[sl].reshape(1, S)),
        }
        in_maps.append({k: v.astype(np.float32, copy=False) for k, v in m.items()})
    return in_maps


def _run(in_maps, trace=False):
    res = run_bass_kernel_spmd(_get_nc(), in_maps, core_ids=list(range(8)),
                               trace=trace)
    return res


def kernel(q, kv, Wq, bq, Wk, bk, Wv, bv, Wo, bo, _trace=False):
    q, kv = np.asarray(q, np.float32), np.asarray(kv, np.float32)
    Wq, Wk = np.asarray(Wq, np.float32), np.asarray(Wk, np.float32)
    Wv, Wo = np.asarray(Wv, np.float32), np.asarray(Wo, np.float32)
    bq, bk = np.asarray(bq, np.float32), np.asarray(bk, np.float32)
    bv, bo = np.asarray(bv, np.float32), np.asarray(bo, np.float32)

    in_maps = _shard(q, kv, Wq, bq, Wk, bk, Wv, bv, Wo, bo)
    res = _run(in_maps, trace=_trace)
    B = q.shape[0]
    outp = np.empty((B, LQ, D), np.float32)
    for b in range(B):
        p0 = res.results[2 * b]["out"].reshape(LQ, D)
        p1 = res.results[2 * b + 1]["out"].reshape(LQ, D)
        outp[b] = p0 + p1 + bo[None, :]
    if _trace:
        kernel._last_exec_ns = res.exec_time_ns
        kernel._last_trace = res.instructions_and_trace
    return outp



# revision 5
# speedup vs baseline: 1.6338x; 1.6338x over previous
"""Cross-attention kernel for TRN2, 8 NeuronCores.

Sharding: core c -> (batch b = c//2, head-group g = c%2).  Each head-group is
8 heads = 512 of the 1024 d_model channels.  All operands bf16 (halves DMA
and lets every matmul run 1 cycle/row at any free size); psum accumulation f32.

Per core (s = 512 shard channels, 4 head-pairs o):
  KT[s, lkv]   = Wk_g^T-contraction over d                    (proj, bf16)
  QT[s, lq]    = (Wq_g/8)^T q                                 (scale folded)
  Vp[lkv,h,65] = kv Wv_h + bias, 65th col = ones (denominator trick)
  ST[lkv, lq]  per head = Kh Qh^T-contraction over dh=64      (psum tile/t)
  P = exp(ST) -> bf16 SBUF                                     (Act engine)
  ctx[lq, 65]  per (head, lq-chunk) = P^T-stationary @ Vp      (free=65!)
                 col 64 = softmax denominator per q partition
  ctx_norm = ctx[:,0:64] * 1/ctx[:,64]                         (DVE)
  cT[s, lq]    = dma-transpose of ctx_norm                     (DMA xbar)
  out[lq, d]  += cT^T @ Wo_g                                   (psum over so)
Host sums the two head-group partials per batch and adds bo.

Emission is software-pipelined: projection work (K/Q/V chunks) is interleaved
as "filler" into the score/exp stream so the PE never idles while the Act
engine chews the 128-tile exp stream; attn@V waves for head-pair o run during
head-pair o+1's score loop (pt pool bufs=4 carries the P tiles across).
"""

import sys
if "/opt/trn_rl_repo" not in sys.path:
    sys.path.insert(0, "/opt/trn_rl_repo")

import ml_dtypes
import numpy as np

import concourse.bass as bass
import concourse.mybir as mybir
import concourse.tile as tile
from concourse.bass_utils import run_bass_kernel_spmd

f32 = mybir.dt.float32
bf16 = mybir.dt.bfloat16
EXP = mybir.ActivationFunctionType.Exp

D = 1024        # d_model
S = 512         # per-core channel shard (8 heads x 64)
LQ = 512
LKV = 2048
CO = D // 128   # 8 contraction chunks
SO = S // 128   # 4 shard s-tiles (head pairs)
NT = LKV // 128  # 16 lkv tiles
NKC = LKV // 512  # 4 lkv 512-chunks


def _split_multi_waits(nc, max_waits=1):
    """This container's walrus allows only `max_waits` sync-wait commands per
    instruction; hoist the excess into standalone EventSemaphore insts."""
    ev_id = 0
    for f in nc.m.functions:
        for bb in f.blocks:
            new = []
            changed = False
            for inst in bb.instructions:
                si = inst.sync_info
                if si is not None and si.on_wait and len(si.on_wait) > max_waits:
                    waits = list(si.on_wait)
                    for sw in waits[:-max_waits]:
                        ev = mybir.InstEventSemaphore(
                            name=f"EVSPLIT-{ev_id}", engine=inst.engine,
                            sync_info=mybir.SyncInfo(on_wait=[sw], on_update=[]))
                        ev_id += 1
                        nc.register_instruction(ev, overwrite=True)
                        new.append(ev)
                    inst.sync_info = mybir.SyncInfo(
                        on_wait=waits[-max_waits:], on_update=list(si.on_update))
                    changed = True
                new.append(inst)
            if changed:
                bb.instructions = new
    return nc


def _build():
    nc = bass.Bass(trn_type="TRN2")

    # DRAM I/O (pre-laid-out on host, bf16 except biases/out)
    qT = nc.dram_tensor("qT", [128, CO, LQ], bf16, kind="ExternalInput")
    kvT = nc.dram_tensor("kvT", [128, NKC, CO, 512], bf16, kind="ExternalInput")
    wqT = nc.dram_tensor("wqT", [128, CO, S], bf16, kind="ExternalInput")
    wkT = nc.dram_tensor("wkT", [128, CO, S], bf16, kind="ExternalInput")
    wvT = nc.dram_tensor("wvT", [128, CO, S], bf16, kind="ExternalInput")
    woT = nc.dram_tensor("woT", [128, SO, D], bf16, kind="ExternalInput")
    bqk = nc.dram_tensor("bqk", [128, 2 * SO], f32, kind="ExternalInput")
    bvb = nc.dram_tensor("bvb", [128, 8, 64], f32, kind="ExternalInput")
    out = nc.dram_tensor("out", [SO, 128, D], f32, kind="ExternalOutput")

    with tile.TileContext(nc) as tc:
        with tc.tile_pool(name="wgt", bufs=1) as wgt, \
             tc.tile_pool(name="big", bufs=1) as big, \
             tc.tile_pool(name="ptp", bufs=4) as ptp, \
             tc.tile_pool(name="sml", bufs=8) as sml, \
             tc.tile_pool(name="ostg", bufs=4) as ostg, \
             tc.tile_pool(name="psS", bufs=2, space="PSUM") as psS, \
             tc.tile_pool(name="psC", bufs=4, space="PSUM") as psC, \
             tc.tile_pool(name="psP", bufs=2, space="PSUM") as psP:

            # ---- resident inputs ----
            kv_sb = wgt.tile([128, NKC, CO, 512], bf16, name="kv_sb")
            wq_sb = wgt.tile([128, CO, S], bf16, name="wq_sb")
            wk_sb = wgt.tile([128, CO, S], bf16, name="wk_sb")
            wv_sb = wgt.tile([128, CO, S], bf16, name="wv_sb")
            wo_sb = wgt.tile([128, SO, D], bf16, name="wo_sb")
            qT_sb = wgt.tile([128, CO, LQ], bf16, name="qT_sb")
            bqk_sb = wgt.tile([128, 2 * SO], f32, name="bqk_sb")
            bvb_sb = wgt.tile([128, 8, 64], f32, name="bvb_sb")

            # ---- resident intermediates ----
            KT_sb = big.tile([128, SO, LKV], bf16, name="KT_sb")    # (s, lkv)
            QT_sb = big.tile([128, SO, LQ], bf16, name="QT_sb")     # (s, lq)
            # V padded per head with a ones column -> attn@V also emits the
            # softmax denominator (psum col 64 per q-partition).
            Vp_sb = big.tile([128, NT, 8, 65], bf16, name="Vp_sb")
            # ctx, normalized, [q-chunk, head, dh]
            ctx_sb = big.tile([128, SO, 8, 64], bf16, name="ctx_sb")
            cT_sb = big.tile([128, SO, LQ], bf16, name="cT_sb")     # (s, lq)

            # ---- DMA loads, ordered by first use ----
            nc.sync.dma_start(bqk_sb, bqk[:])
            nc.sync.dma_start(wk_sb, wkT[:])
            nc.sync.dma_start(kv_sb[:, 0], kvT[:, 0])
            nc.sync.dma_start(wq_sb, wqT[:])
            nc.sync.dma_start(qT_sb, qT[:])
            nc.sync.dma_start(wv_sb, wvT[:])
            nc.sync.dma_start(bvb_sb, bvb[:])
            for ch in range(1, NKC):
                nc.sync.dma_start(kv_sb[:, ch], kvT[:, ch])
            nc.sync.dma_start(wo_sb, woT[:])

            # ones column of Vp (denominator trick)
            nc.gpsimd.memset(Vp_sb[:, :, :, 64:65], 1.0)

            # ---- work-chunk emitters (each ~1.7us of PE) ----
            def k_chunk(o, ch):
                ps = psP.tile([128, 512], f32, name="kps", tag="proj")
                for c in range(CO):
                    nc.tensor.matmul(
                        ps, wk_sb[:, c, o * 128:(o + 1) * 128],
                        kv_sb[:, ch, c, :], start=(c == 0), stop=(c == CO - 1))
                nc.vector.tensor_scalar_add(
                    KT_sb[:, o, ch * 512:(ch + 1) * 512], ps,
                    bqk_sb[:, SO + o:SO + o + 1])

            def q_chunk(so):
                ps = psP.tile([128, 512], f32, name="qps", tag="proj")
                for c in range(CO):
                    nc.tensor.matmul(
                        ps, wq_sb[:, c, so * 128:(so + 1) * 128],
                        qT_sb[:, c, :], start=(c == 0), stop=(c == CO - 1))
                nc.vector.tensor_scalar_add(
                    QT_sb[:, so, :], ps, bqk_sb[:, so:so + 1])

            def v_chunk(t):
                ps = psP.tile([128, 512], f32, name="vps", tag="proj")
                ch, tt = t // 4, t % 4
                for c in range(CO):
                    nc.tensor.matmul(
                        ps, kv_sb[:, ch, c, tt * 128:(tt + 1) * 128],
                        wv_sb[:, c, :], start=(c == 0), stop=(c == CO - 1))
                nc.vector.tensor_add(
                    Vp_sb[:, t, :, 0:64],
                    ps.rearrange("p (h d) -> p h d", h=8), bvb_sb)

            # ---- attention state ----
            pt_tiles = {}   # (o, h) -> [128, NT, 512] bf16 P^T tiles
            ctx_ps = {}     # (o, qc, h) -> [128, 65] psum

            def scores(o, t):
                stA = psS.tile([128, 512], f32, name="stA", tag="sc")
                stB = psS.tile([128, 512], f32, name="stB", tag="sc")
                nc.tensor.matmul(stA, KT_sb[0:64, o, t * 128:(t + 1) * 128],
                                 QT_sb[0:64, o, :], start=True, stop=True)
                nc.tensor.matmul(stB, KT_sb[64:128, o, t * 128:(t + 1) * 128],
                                 QT_sb[64:128, o, :], start=True, stop=True)
                nc.scalar.activation(pt_tiles[(o, 0)][:, t, :], stA, EXP)
                nc.scalar.activation(pt_tiles[(o, 1)][:, t, :], stB, EXP)

            def av_quarter(o, qc, phase):
                # 8 mms: lkv tiles 4*phase..4*phase+3 for both heads of pair o
                if phase == 0:
                    for h in (0, 1):
                        ctx_ps[(o, qc, h)] = psC.tile(
                            [128, 65], f32, name=f"ctx{h}", tag="ctx")
                for tp in range(4 * phase, 4 * phase + 4):
                    for h in (0, 1):
                        nc.tensor.matmul(
                            ctx_ps[(o, qc, h)],
                            pt_tiles[(o, h)][:, tp, qc * 128:(qc + 1) * 128],
                            Vp_sb[:, tp, 2 * o + h, :],
                            start=(tp == 0), stop=(tp == NT - 1))

            def norm_t(o, qc):
                # normalize both heads of (o, qc), then dma-transpose the
                # [q,128]x[2*64] block into cT
                for h in (0, 1):
                    ctx = ctx_ps.pop((o, qc, h))
                    rcp = sml.tile([128, 1], f32, name="rcp", tag="rcp")
                    nc.vector.reciprocal(rcp, ctx[:, 64:65])
                    nc.vector.tensor_scalar_mul(
                        ctx_sb[:, qc, 2 * o + h, :], ctx[:, 0:64], rcp)
                nc.sync.dma_start_transpose(
                    cT_sb[:, o, qc * 128:(qc + 1) * 128],
                    ctx_sb[:, qc, 2 * o:2 * o + 2, :])

            def out_proj(lt):
                for dc in range(2):
                    ps = psS.tile([128, 512], f32, name="ops", tag="sc")
                    for so in range(SO):
                        nc.tensor.matmul(
                            ps, cT_sb[:, so, lt * 128:(lt + 1) * 128],
                            wo_sb[:, so, dc * 512:(dc + 1) * 512],
                            start=(so == 0), stop=(so == SO - 1))
                    ot = ostg.tile([128, 512], f32, name="ot", tag="ot")
                    if dc == 0:
                        nc.scalar.copy(ot, ps)
                    else:
                        nc.vector.tensor_copy(ot, ps)
                    nc.sync.dma_start(out[lt, :, dc * 512:(dc + 1) * 512], ot)

            # ---- fillers per score-loop, ordered to match DMA arrivals ----
            fillers = {
                0: [lambda: q_chunk(1), lambda: k_chunk(1, 0),
                    lambda: v_chunk(0), lambda: v_chunk(1),
                    lambda: q_chunk(2), lambda: k_chunk(0, 1),
                    lambda: v_chunk(2), lambda: v_chunk(3),
                    lambda: q_chunk(3), lambda: k_chunk(1, 1),
                    lambda: v_chunk(4), lambda: v_chunk(5),
                    lambda: k_chunk(0, 2), lambda: k_chunk(1, 2),
                    lambda: v_chunk(6), lambda: v_chunk(7),
                    lambda: v_chunk(8), lambda: v_chunk(9),
                    lambda: k_chunk(0, 3), lambda: k_chunk(1, 3),
                    lambda: v_chunk(10), lambda: v_chunk(11),
                    lambda: v_chunk(12), lambda: v_chunk(13),
                    lambda: v_chunk(14), lambda: v_chunk(15)],
                1: [lambda: k_chunk(2, 0), lambda: k_chunk(2, 1),
                    lambda: k_chunk(2, 2), lambda: k_chunk(2, 3)],
                2: [lambda: k_chunk(3, 0), lambda: k_chunk(3, 1),
                    lambda: k_chunk(3, 2), lambda: k_chunk(3, 3)],
                3: [],
            }

            # ---- head: first K chunk + first Q chunk ----
            k_chunk(0, 0)
            q_chunk(0)

            # ---- main pipelined loops ----
            for o in range(SO):
                pt_tiles[(o, 0)] = ptp.tile([128, NT, 512], bf16,
                                            name="ptA", tag="pt")
                pt_tiles[(o, 1)] = ptp.tile([128, NT, 512], bf16,
                                            name="ptB", tag="pt")
                fl = fillers[o]
                pumped = 0
                for t in range(NT):
                    scores(o, t)
                    if o >= 1:
                        qc, phase = divmod(t, 4)
                        av_quarter(o - 1, qc, phase)
                        if phase == 3:
                            norm_t(o - 1, qc)
                    # pump fillers, spread evenly across the 16 iterations
                    want = ((t + 1) * len(fl) + NT - 1) // NT
                    while pumped < want:
                        fl[pumped]()
                        pumped += 1

            # ---- tail: AV(3) waves, normalize, transpose, out-proj ----
            # qc0 was NOT trailed above (kept simple); do all 4 waves here.
            for qc in range(SO):
                for phase in range(4):
                    av_quarter(3, qc, phase)
                norm_t(3, qc)
                if qc >= 1:
                    out_proj(qc - 1)
            out_proj(3)

    return _split_multi_waits(nc)


_NC = None


def _get_nc():
    global _NC
    if _NC is None:
        _NC = _build()
    return _NC


def _shard(q, kv, Wq, bq, Wk, bk, Wv, bv, Wo, bo):
    bf = ml_dtypes.bfloat16

    def lay(a2d, co):  # [co*128, F] -> [128, co, F] bf16
        F = a2d.shape[1]
        return np.ascontiguousarray(
            a2d.reshape(co, 128, F).transpose(1, 0, 2)).astype(bf)

    in_maps = []
    for core in range(8):
        b, g = core // 2, core % 2
        sl = slice(g * S, (g + 1) * S)
        kvt = lay(np.ascontiguousarray(kv[b].T), CO)  # [128, 8, 2048]
        kvt = np.ascontiguousarray(
            kvt.reshape(128, CO, NKC, 512).transpose(0, 2, 1, 3))
        bqk_arr = np.concatenate([
            (bq[sl] * 0.125).reshape(SO, 128).T,
            bk[sl].reshape(SO, 128).T], axis=1)
        m = {
            "qT": lay(np.ascontiguousarray(q[b].T), CO),
            "kvT": kvt,
            "wqT": lay(np.ascontiguousarray((Wq[sl] * 0.125).T), CO),
            "wkT": lay(np.ascontiguousarray(Wk[sl].T), CO),
            "wvT": lay(np.ascontiguousarray(Wv[sl].T), CO),
            "woT": lay(np.ascontiguousarray(Wo[:, sl].T), SO),
            "bqk": np.ascontiguousarray(bqk_arr, dtype=np.float32),
            "bvb": np.ascontiguousarray(
                np.broadcast_to(bv[sl].reshape(1, 8, 64), (128, 8, 64)),
                dtype=np.float32),
        }
        in_maps.append(m)
    return in_maps


def _run(in_maps, trace=False):
    res = run_bass_kernel_spmd(_get_nc(), in_maps, core_ids=list(range(8)),
                               trace=trace)
    return res


def kernel(q, kv, Wq, bq, Wk, bk, Wv, bv, Wo, bo, _trace=False):
    q, kv = np.asarray(q, np.float32), np.asarray(kv, np.float32)
    Wq, Wk = np.asarray(Wq, np.float32), np.asarray(Wk, np.float32)
    Wv, Wo = np.asarray(Wv, np.float32), np.asarray(Wo, np.float32)
    bq, bk = np.asarray(bq, np.float32), np.asarray(bk, np.float32)
    bv, bo = np.asarray(bv, np.float32), np.asarray(bo, np.float32)

    in_maps = _shard(q, kv, Wq, bq, Wk, bk, Wv, bv, Wo, bo)
    res = _run(in_maps, trace=_trace)
    B = q.shape[0]
    outp = np.empty((B, LQ, D), np.float32)
    for b in range(B):
        p0 = res.results[2 * b]["out"].reshape(LQ, D)
        p1 = res.results[2 * b + 1]["out"].reshape(LQ, D)
        outp[b] = p0 + p1 + bo[None, :]
    if _trace:
        kernel._last_exec_ns = res.exec_time_ns
        kernel._last_trace = res.instructions_and_trace
    return outp


# revision 20
# speedup vs baseline: 1.7998x; 1.1016x over previous
"""Cross-attention kernel for TRN2, 8 NeuronCores.

Sharding: core c -> (batch b = c//2, head-group g = c%2).  Each head-group is
8 heads = 512 of the 1024 d_model channels.  All operands bf16 (halves DMA
and lets every matmul run 1 cycle/row at any free size); psum accumulation f32.

Per core (s = 512 shard channels, 4 head-pairs o):
  KT[s, lkv]   = Wk_g^T-contraction over d                    (proj, bf16)
  QT[s, lq]    = (Wq_g/8)^T q                                 (scale folded)
  Vp[lkv,h,65] = kv Wv_h + bias, 65th col = ones (denominator trick)
  ST[lkv, lq]  per head = Kh Qh^T-contraction over dh=64      (psum tile/t)
  P = exp(ST) -> bf16 SBUF                                     (Act engine)
  ctx[lq, 65]  per (head, lq-chunk) = P^T-stationary @ Vp      (free=65!)
                 col 64 = softmax denominator per q partition
  ctx_norm = ctx[:,0:64] * 1/ctx[:,64]                         (DVE)
  cT[s, lq]    = dma-transpose of ctx_norm                     (DMA xbar)
  out[lq, d]  += cT^T @ Wo_g                                   (psum over so)
Host sums the two head-group partials per batch and adds bo.

Emission is software-pipelined: projection work (K/Q/V chunks) is interleaved
as "filler" into the score/exp stream so the PE never idles while the Act
engine chews the 128-tile exp stream; attn@V waves for head-pair o run during
head-pair o+1's score loop (pt pool bufs=4 carries the P tiles across).
"""

import sys
if "/opt/trn_rl_repo" not in sys.path:
    sys.path.insert(0, "/opt/trn_rl_repo")

import ml_dtypes
import numpy as np

import concourse.bass as bass
import concourse.mybir as mybir
import concourse.tile as tile
from concourse.bass_utils import run_bass_kernel_spmd

f32 = mybir.dt.float32
bf16 = mybir.dt.bfloat16
EXP = mybir.ActivationFunctionType.Exp

D = 1024        # d_model
S = 512         # per-core channel shard (8 heads x 64)
LQ = 512
LKV = 2048
CO = D // 128   # 8 contraction chunks
SO = S // 128   # 4 shard s-tiles (head pairs)
NT = LKV // 128  # 16 lkv tiles
NKC = LKV // 512  # 4 lkv 512-chunks


def _split_multi_waits(nc, max_waits=1):
    """This container's walrus allows only `max_waits` sync-wait commands per
    instruction; hoist the excess into standalone EventSemaphore insts."""
    ev_id = 0
    for f in nc.m.functions:
        for bb in f.blocks:
            new = []
            changed = False
            for inst in bb.instructions:
                si = inst.sync_info
                if si is not None and si.on_wait and len(si.on_wait) > max_waits:
                    waits = list(si.on_wait)
                    for sw in waits[:-max_waits]:
                        ev = mybir.InstEventSemaphore(
                            name=f"EVSPLIT-{ev_id}", engine=inst.engine,
                            sync_info=mybir.SyncInfo(on_wait=[sw], on_update=[]))
                        ev_id += 1
                        nc.register_instruction(ev, overwrite=True)
                        new.append(ev)
                    inst.sync_info = mybir.SyncInfo(
                        on_wait=waits[-max_waits:], on_update=list(si.on_update))
                    changed = True
                new.append(inst)
            if changed:
                bb.instructions = new
    return nc


def _build():
    nc = bass.Bass(trn_type="TRN2")

    # DRAM I/O (pre-laid-out on host, bf16 except biases/out)
    qT = nc.dram_tensor("qT", [128, CO, LQ], bf16, kind="ExternalInput")
    kvT = nc.dram_tensor("kvT", [128, NKC, CO, 512], bf16, kind="ExternalInput")
    wqT = nc.dram_tensor("wqT", [128, SO, CO, 128], bf16, kind="ExternalInput")
    wkT = nc.dram_tensor("wkT", [128, SO, CO, 128], bf16, kind="ExternalInput")
    wvT = nc.dram_tensor("wvT", [128, CO, S], bf16, kind="ExternalInput")
    woT = nc.dram_tensor("woT", [128, SO, D], bf16, kind="ExternalInput")
    bqk = nc.dram_tensor("bqk", [128, 2 * SO], f32, kind="ExternalInput")
    bvb = nc.dram_tensor("bvb", [128, 8, 64], f32, kind="ExternalInput")
    out = nc.dram_tensor("out", [SO, 128, D], f32, kind="ExternalOutput")

    with tile.TileContext(nc) as tc:
        with tc.tile_pool(name="wgt", bufs=1) as wgt, \
             tc.tile_pool(name="big", bufs=1) as big, \
             tc.tile_pool(name="ptp", bufs=4) as ptp, \
             tc.tile_pool(name="sml", bufs=8) as sml, \
             tc.tile_pool(name="ostg", bufs=4) as ostg, \
             tc.tile_pool(name="psS", bufs=2, space="PSUM") as psS, \
             tc.tile_pool(name="psC", bufs=2, space="PSUM") as psC, \
             tc.tile_pool(name="psP", bufs=2, space="PSUM") as psP:

            # ---- resident inputs ----
            kv_sb = wgt.tile([128, NKC, CO, 512], bf16, name="kv_sb")
            wq_sb = wgt.tile([128, SO, CO, 128], bf16, name="wq_sb")
            wk_sb = wgt.tile([128, SO, CO, 128], bf16, name="wk_sb")
            wv_sb = wgt.tile([128, CO, S], bf16, name="wv_sb")
            wo_sb = wgt.tile([128, SO, D], bf16, name="wo_sb")
            qT_sb = wgt.tile([128, CO, LQ], bf16, name="qT_sb")
            bqk_sb = wgt.tile([128, 2 * SO], f32, name="bqk_sb")
            bvb_sb = wgt.tile([128, 8, 64], f32, name="bvb_sb")

            # ---- resident intermediates ----
            KT_sb = big.tile([128, SO, LKV], bf16, name="KT_sb")    # (s, lkv)
            QT_sb = big.tile([128, SO, LQ], bf16, name="QT_sb")     # (s, lq)
            # V padded per head with a ones column -> attn@V also emits the
            # softmax denominator (psum col 64 per q-partition).
            Vp_sb = big.tile([128, NT, 8, 65], bf16, name="Vp_sb")
            # ctx, normalized, [q-chunk, head, dh]
            ctx_sb = big.tile([128, SO, 8, 64], bf16, name="ctx_sb")
            cT_sb = big.tile([128, SO, LQ], bf16, name="cT_sb")     # (s, lq)

            # ---- DMA loads, ordered by first use; head splits so K(0,0)
            # ---- and Q(0) start as early as possible ----
            nc.sync.dma_start(bqk_sb, bqk[:])
            nc.sync.dma_start(wk_sb[:, 0], wkT[:, 0])
            nc.sync.dma_start(kv_sb[:, 0, 0:4], kvT[:, 0, 0:4])
            nc.sync.dma_start(kv_sb[:, 0, 4:8], kvT[:, 0, 4:8])
            nc.sync.dma_start(wq_sb[:, 0], wqT[:, 0])
            nc.sync.dma_start(qT_sb[:, 0:4], qT[:, 0:4])
            nc.sync.dma_start(qT_sb[:, 4:8], qT[:, 4:8])
            nc.sync.dma_start(kv_sb[:, 1], kvT[:, 1])
            nc.sync.dma_start(wk_sb[:, 1:4], wkT[:, 1:4])
            nc.sync.dma_start(bvb_sb, bvb[:])
            nc.sync.dma_start(wv_sb, wvT[:])
            nc.sync.dma_start(kv_sb[:, 2], kvT[:, 2])
            nc.sync.dma_start(wq_sb[:, 1:4], wqT[:, 1:4])
            nc.sync.dma_start(kv_sb[:, 3], kvT[:, 3])
            nc.sync.dma_start(wo_sb, woT[:])

            # ones column of Vp (denominator trick)
            nc.gpsimd.memset(Vp_sb[:, :, :, 64:65], 1.0)

            # ---- work-chunk emitters (each ~1.7us of PE) ----
            def k_chunk(o, ch):
                ps = psP.tile([128, 512], f32, name="kps", tag="proj")
                for c in range(CO):
                    nc.tensor.matmul(
                        ps, wk_sb[:, o, c, :],
                        kv_sb[:, ch, c, :], start=(c == 0), stop=(c == CO - 1))
                nc.vector.tensor_scalar_add(
                    KT_sb[:, o, ch * 512:(ch + 1) * 512], ps,
                    bqk_sb[:, SO + o:SO + o + 1])

            def q_chunk(so):
                ps = psP.tile([128, 512], f32, name="qps", tag="proj")
                for c in range(CO):
                    nc.tensor.matmul(
                        ps, wq_sb[:, so, c, :],
                        qT_sb[:, c, :], start=(c == 0), stop=(c == CO - 1))
                nc.vector.tensor_scalar_add(
                    QT_sb[:, so, :], ps, bqk_sb[:, so:so + 1])

            def v_chunk(t):
                ps = psP.tile([128, 512], f32, name="vps", tag="proj")
                ch, tt = t // 4, t % 4
                for c in range(CO):
                    nc.tensor.matmul(
                        ps, kv_sb[:, ch, c, tt * 128:(tt + 1) * 128],
                        wv_sb[:, c, :], start=(c == 0), stop=(c == CO - 1))
                nc.vector.tensor_add(
                    Vp_sb[:, t, :, 0:64],
                    ps.rearrange("p (h d) -> p h d", h=8), bvb_sb)

            # ---- attention state ----
            pt_tiles = {}   # (o, h) -> [128, NT, 512] bf16 P^T tiles
            ctx_ps = {}     # (o, h) -> [128, 4, 65] psum (4 q-chunks, 1 bank)

            def scores2(o, s):
                # two lkv tiles (t=2s, 2s+1) per head; one exp instruction
                # per head covers both tiles (1024-wide, halves Act overhead)
                stA = psS.tile([128, 2, 512], f32, name="stA", tag="sc")
                stB = psS.tile([128, 2, 512], f32, name="stB", tag="sc")
                for j in range(2):
                    t = 2 * s + j
                    nc.tensor.matmul(stA[:, j, :],
                                     KT_sb[0:64, o, t * 128:(t + 1) * 128],
                                     QT_sb[0:64, o, :], start=True, stop=True)
                    nc.tensor.matmul(stB[:, j, :],
                                     KT_sb[64:128, o, t * 128:(t + 1) * 128],
                                     QT_sb[64:128, o, :], start=True, stop=True)
                nc.scalar.activation(
                    pt_tiles[(o, 0)][:, 2 * s:2 * s + 2, :], stA, EXP)
                nc.scalar.activation(
                    pt_tiles[(o, 1)][:, 2 * s:2 * s + 2, :], stB, EXP)

            def av_phase(o, p):
                # 32 mms: lkv tiles 4p..4p+3, all 4 q-chunks, both heads.
                # All 4 q-chunk accumulators of one head share a single psum
                # bank; only the very first mm uses start=True (the psum
                # zero-region covers the whole bank), everything else
                # accumulates with start=False.
                for h in (0, 1):
                    if p == 0:
                        ctx_ps[(o, h)] = psC.tile(
                            [128, 4, 65], f32, name=f"ctx{h}", tag="ctx")
                    ctx = ctx_ps[(o, h)]
                    for qc in range(4):
                        for tp in range(4 * p, 4 * p + 4):
                            nc.tensor.matmul(
                                ctx[:, qc, :],
                                pt_tiles[(o, h)][:, tp, qc * 128:(qc + 1) * 128],
                                Vp_sb[:, tp, 2 * o + h, :],
                                start=(p == 0 and qc == 0 and tp == 0),
                                stop=(p == 3 and tp == NT - 1),
                                skip_group_check=True)

            def norm_o(o):
                # normalize all 8 q-chunk/head blocks of pair o, then
                # dma-transpose the four [q,128]x[128] blocks into cT
                for h in (0, 1):
                    ctx = ctx_ps.pop((o, h))
                    rcp = sml.tile([128, 4, 1], f32, name="rcp", tag="rcp")
                    nc.vector.reciprocal(rcp, ctx[:, :, 64:65])
                    for qc in range(4):
                        nc.vector.tensor_scalar_mul(
                            ctx_sb[:, qc, 2 * o + h, :], ctx[:, qc, 0:64],
                            rcp[:, qc, :])
                for qc in range(4):
                    nc.sync.dma_start_transpose(
                        cT_sb[:, o, qc * 128:(qc + 1) * 128],
                        ctx_sb[:, qc, 2 * o:2 * o + 2, :])

            def out_proj(lt):
                for dc in range(2):
                    ps = psP.tile([128, 512], f32, name="ops", tag="proj")
                    for so in range(SO):
                        nc.tensor.matmul(
                            ps, cT_sb[:, so, lt * 128:(lt + 1) * 128],
                            wo_sb[:, so, dc * 512:(dc + 1) * 512],
                            start=(so == 0), stop=(so == SO - 1))
                    ot = ostg.tile([128, 512], f32, name="ot", tag="ot")
                    if dc == 0:
                        nc.scalar.copy(ot, ps)
                    else:
                        nc.vector.tensor_copy(ot, ps)
                    nc.sync.dma_start(out[lt, :, dc * 512:(dc + 1) * 512], ot)

            # ---- fillers per (o, step), ordered to match DMA arrivals and
            # ---- the just-in-time needs of scores/AV ----
            K, Q, V = k_chunk, q_chunk, v_chunk
            fillers = {
                0: {0: [lambda: K(0, 1)],
                    1: [lambda: K(1, 0)],
                    2: [lambda: K(1, 1), lambda: V(0)],
                    3: [lambda: V(1), lambda: V(2), lambda: V(3),
                        lambda: K(0, 2)],
                    4: [lambda: V(4), lambda: V(5), lambda: K(1, 2)],
                    5: [lambda: V(6), lambda: V(7), lambda: K(0, 3)],
                    6: [lambda: V(8), lambda: V(9), lambda: K(1, 3)],
                    7: [lambda: V(10), lambda: V(11), lambda: Q(1)]},
                1: {0: [lambda: V(12), lambda: V(13), lambda: V(14),
                        lambda: V(15)],
                    1: [lambda: K(2, 0)],
                    3: [lambda: K(2, 1), lambda: Q(2)],
                    5: [lambda: K(2, 2)],
                    7: [lambda: K(2, 3), lambda: Q(3)]},
                2: {1: [lambda: K(3, 0)],
                    3: [lambda: K(3, 1)],
                    5: [lambda: K(3, 2)],
                    7: [lambda: K(3, 3)]},
                3: {},
            }

            # ---- head: first K chunk + first Q chunk ----
            k_chunk(0, 0)
            q_chunk(0)

            # ---- main pipelined loops: 8 steps of 2 lkv tiles each ----
            for o in range(SO):
                pt_tiles[(o, 0)] = ptp.tile([128, NT, 512], bf16,
                                            name="ptA", tag="pt")
                pt_tiles[(o, 1)] = ptp.tile([128, NT, 512], bf16,
                                            name="ptB", tag="pt")
                fl = fillers[o]
                for s in range(8):
                    scores2(o, s)
                    for thunk in fl.get(s, ()):
                        thunk()
                    # AV trails its own exp stream; the last phase (needs
                    # exp of t=15) lands in the next loop / tail
                    if s in (3, 5, 7):
                        av_phase(o, (s - 3) // 2)
                    if o >= 1:
                        if s == 1:
                            av_phase(o - 1, 3)
                        if s == 2:
                            norm_o(o - 1)

            # ---- tail: last AV phase, normalize, transpose, out-proj ----
            av_phase(3, 3)
            norm_o(3)
            for lt in range(SO):
                out_proj(lt)

    return _split_multi_waits(nc)


_NC = None


def _get_nc():
    global _NC
    if _NC is None:
        _NC = _build()
    return _NC


def _shard(q, kv, Wq, bq, Wk, bk, Wv, bv, Wo, bo):
    bf = ml_dtypes.bfloat16

    def lay(a2d, co):  # [co*128, F] -> [128, co, F] bf16
        F = a2d.shape[1]
        return np.ascontiguousarray(
            a2d.reshape(co, 128, F).transpose(1, 0, 2)).astype(bf)

    in_maps = []
    for core in range(8):
        b, g = core // 2, core % 2
        sl = slice(g * S, (g + 1) * S)
        kvt = lay(np.ascontiguousarray(kv[b].T), CO)  # [128, 8, 2048]
        kvt = np.ascontiguousarray(
            kvt.reshape(128, CO, NKC, 512).transpose(0, 2, 1, 3))
        bqk_arr = np.concatenate([
            (bq[sl] * 0.125).reshape(SO, 128).T,
            bk[sl].reshape(SO, 128).T], axis=1)

        def lay4(w):  # [S, D] -> [128, SO, CO, 128] (per-so contiguous)
            a = lay(np.ascontiguousarray(w.T), CO)  # [128, CO, S]
            return np.ascontiguousarray(
                a.reshape(128, CO, SO, 128).transpose(0, 2, 1, 3))

        m = {
            "qT": lay(np.ascontiguousarray(q[b].T), CO),
            "kvT": kvt,
            "wqT": lay4(Wq[sl] * 0.125),
            "wkT": lay4(Wk[sl]),
            "wvT": lay(np.ascontiguousarray(Wv[sl].T), CO),
            "woT": lay(np.ascontiguousarray(Wo[:, sl].T), SO),
            "bqk": np.ascontiguousarray(bqk_arr, dtype=np.float32),
            "bvb": np.ascontiguousarray(
                np.broadcast_to(bv[sl].reshape(1, 8, 64), (128, 8, 64)),
                dtype=np.float32),
        }
        in_maps.append(m)
    return in_maps


def _run(in_maps, trace=False):
    res = run_bass_kernel_spmd(_get_nc(), in_maps, core_ids=list(range(8)),
                               trace=trace)
    return res


def kernel(q, kv, Wq, bq, Wk, bk, Wv, bv, Wo, bo, _trace=False):
    q, kv = np.asarray(q, np.float32), np.asarray(kv, np.float32)
    Wq, Wk = np.asarray(Wq, np.float32), np.asarray(Wk, np.float32)
    Wv, Wo = np.asarray(Wv, np.float32), np.asarray(Wo, np.float32)
    bq, bk = np.asarray(bq, np.float32), np.asarray(bk, np.float32)
    bv, bo = np.asarray(bv, np.float32), np.asarray(bo, np.float32)

    in_maps = _shard(q, kv, Wq, bq, Wk, bk, Wv, bv, Wo, bo)
    res = _run(in_maps, trace=_trace)
    B = q.shape[0]
    outp = np.empty((B, LQ, D), np.float32)
    for b in range(B):
        p0 = res.results[2 * b]["out"].reshape(LQ, D)
        p1 = res.results[2 * b + 1]["out"].reshape(LQ, D)
        outp[b] = p0 + p1 + bo[None, :]
    if _trace:
        kernel._last_exec_ns = res.exec_time_ns
        kernel._last_trace = res.instructions_and_trace
    return outp
